# revision 15
# baseline (speedup 1.0000x reference)
"""Trainium2 Bass kernel for nn_CantorModalityFusion.

Sharding: 8 cores = (batch b in 0..3) x (position parity in 0..1).
Each core handles batch b, positions s = par, par+2, ... (1024 positions).
The computation is per-(b, s) independent -> no collectives.

v2: the per-modality input projection is folded into the QKV weights on
the host (Wf = Wp @ W_m, beta = Wp @ (b_m + emb_m) + b_p), so the device
computes q/k/v for each modality directly from the raw modality input:
  q_m = Wfq_m @ x_m + beta  (contraction over dim_m instead of D)
This removes the separate projection stage and cuts PE work ~35%.
Weights and x stream in bf16 (fp32 PSUM accumulation); q/k/v, scores,
softmax, fused accumulation and the output projection stay fp32.

Pipeline per 512-position block:
  B1: q.T/k.T per feature chunk from x; s_w += sel.T @ (q*k)     [PE+DVE]
  SM: softmax over the 3 routed windows                          [DVE+ACT]
  B2: v.T per chunk; A16_r = sum attn; Abc = bcast(A16);
      fused.T[c] = sum_r Abc_r * v.T[r]                          [PE+DVE]
  D:  y.T = Wo.T.T @ fused.T (+ bo)  (deferred into next block)  [PE+ACT]
"""

import sys

import numpy as np

sys.path.insert(0, "/opt/trn_rl_repo")

import ml_dtypes

import concourse.bacc as bacc
import concourse.mybir as mybir
from concourse import tile
from concourse.bass_utils import run_bass_kernel_spmd

F32 = mybir.dt.float32
F32R = mybir.dt.float32r
BF16 = mybir.dt.bfloat16
AF = mybir.ActivationFunctionType
ALU = mybir.AluOpType

B, S, D, H, HD = 4, 2048, 1024, 16, 64
M, WIN = 4, 3
MOD = [("text", 768, 2048), ("image", 1024, 1024), ("audio", 512, 1500), ("video", 2048, 512)]
ROUTES = [[0, 1, 2], [0, 1, 2], [2, 3, 0], [3, 2, 0]]
PAIRS = [(m, w, ROUTES[m][w]) for m in range(M) for w in range(WIN)]
SRC = {r: [(m, w) for (m, w, rr) in PAIRS if rr == r] for r in range(M)}
PAIR_IDX = {(m, w): m * WIN + w for m in range(M) for w in range(WIN)}

NPOS = S // 2
BLK = 512
NBLK = NPOS // BLK
NCH = D // 128                           # 8 output feature chunks
NLOC = [sl // 2 for (_, _, sl) in MOD]   # 1024, 512, 750, 256
NK = [dim // 128 for (_, dim, _) in MOD]  # 6, 8, 4, 16 input chunks

_BUILD_CACHE = {}


def n_active(m, blk):
    return max(0, min(BLK, NLOC[m] - blk * BLK))


def build(scale, repeat=1):
    key = (float(scale), repeat)
    if key in _BUILD_CACHE:
        return _BUILD_CACHE[key]
    nc = bacc.Bacc("TRN2", target_bir_lowering=False, debug=False)

    xT = [nc.dram_tensor(f"xT{m}", [128, NK[m], NLOC[m]], BF16,
                         kind="ExternalInput") for m in range(M)]
    # fused q/k weights, merged over the modalities active in each block:
    # [c, p(din%128), ti(q/k), dk(concat over act_m), j(dout%128)]
    SNK = [sum(NK[m] for m in range(M) if n_active(m, b) > 0)
           for b in range(NBLK)]
    Wqk = [nc.dram_tensor(f"Wqk{b}", [NCH, 128, 2, SNK[b], 128], BF16,
                          kind="ExternalInput") for b in range(NBLK)]
    Wvf = [nc.dram_tensor(f"Wvf{b}", [NCH, 128, SNK[b], 128], BF16,
                          kind="ExternalInput") for b in range(NBLK)]
    WoT = nc.dram_tensor("WoT", [128, NCH, NCH, 128], BF16, kind="ExternalInput")
    betaqk_d = nc.dram_tensor("betaqk", [128, M, 2, NCH], F32, kind="ExternalInput")
    betav_d = nc.dram_tensor("betav", [128, M, NCH], F32, kind="ExternalInput")
    bo_d = nc.dram_tensor("bo", [128, NCH], F32, kind="ExternalInput")
    selw_d = nc.dram_tensor("selw", [128, 127], BF16, kind="ExternalInput")
    selA_d = nc.dram_tensor("selA", [64, M * WIN, 16], BF16, kind="ExternalInput")
    selB_d = nc.dram_tensor("selB", [16, NCH, 128], BF16, kind="ExternalInput")
    yT = nc.dram_tensor("yT", [128, NCH, NPOS], F32, kind="ExternalOutput")

    with tile.TileContext(nc) as tc:
        with (
            tc.tile_pool(name="const", bufs=1) as cpool,
            tc.tile_pool(name="wq", bufs=2) as wqpool,
            tc.tile_pool(name="wo", bufs=2) as wopool,
            tc.tile_pool(name="xt", bufs=1) as xtpool,
            tc.tile_pool(name="qk", bufs=1) as qkpool,
            tc.tile_pool(name="pr", bufs=2) as prpool,
            tc.tile_pool(name="sm", bufs=1) as smpool,
            tc.tile_pool(name="fz", bufs=2) as fzpool,
            tc.tile_pool(name="yo", bufs=2) as yopool,
            tc.tile_pool(name="ps", bufs=1, space="PSUM") as pspool,
        ):
            def psum(i, shape=(128, BLK)):
                return pspool.tile(list(shape), F32, tag=f"a{i}", name=f"ps_a{i}")

            # ---- constants ----
            selw = cpool.tile([128, 127], BF16, tag="selw")
            nc.sync.dma_start(selw[:], selw_d[:])
            selA = cpool.tile([64, M * WIN, 16], BF16, tag="selA")
            nc.sync.dma_start(selA[:], selA_d[:])
            selB = cpool.tile([16, NCH, 128], BF16, tag="selB")
            nc.sync.dma_start(selB[:], selB_d[:])
            betaqk = cpool.tile([128, M, 2, NCH], F32, tag="betaqk")
            nc.sync.dma_start(betaqk[:], betaqk_d[:])
            betav = cpool.tile([128, M, NCH], F32, tag="betav")
            nc.sync.dma_start(betav[:], betav_d[:])
            bo = cpool.tile([128, NCH], F32, tag="bo")
            nc.sync.dma_start(bo[:], bo_d[:])

            import contextlib
            rep_cm = (tc.For_i(0, repeat, 1,
                               hint_engines=(mybir.EngineType.PE,
                                             mybir.EngineType.Activation,
                                             mybir.EngineType.DVE,
                                             mybir.EngineType.SP,
                                             mybir.EngineType.Pool))
                      if repeat > 1 else contextlib.nullcontext())
            pending_D = []
            with rep_cm:
                for blk in range(NBLK):
                    nact = [n_active(m, blk) for m in range(M)]
                    act_m = [m for m in range(M) if nact[m] > 0]
                    act_pairs = [(m, w, r) for (m, w, r) in PAIRS
                                 if nact[m] > 0 and nact[r] > 0]
                    p0 = blk * BLK

                    # ---------- x tiles (bf16, reused by q, k and v) ----------
                    xt = {}
                    dkoff = {}
                    off = 0
                    for mi, m in enumerate(act_m):
                        na = nact[m]
                        dkoff[m] = off
                        off += NK[m]
                        t = xtpool.tile([128, NK[m], BLK], BF16,
                                        tag=f"x{m}_{blk % 2}")
                        nc.scalar.dma_start(t[:, :, :na], xT[m][:, :, p0:p0 + na])
                        for dk in range(NK[m]):
                            xt[(m, dk)] = t[:, dk, :]

                    # ---------- pass 1: q, k, scores ----------
                    n_sc = {w: sum(1 for (m, w2, r) in act_pairs if w2 == w) * NCH
                            for w in range(WIN)}
                    c_sc = {w: 0 for w in range(WIN)}
                    sc_ps = [psum(5 + w, (64, BLK)) if n_sc[w] > 0 else None
                             for w in range(WIN)]
                    qk_ctr = [0]

                    def emit_qk(c):
                        qk_t = {}
                        wqk = wqpool.tile([128, 2, SNK[blk], 128], BF16,
                                          tag=f"wqk{blk % 2}", name="wqk")
                        nc.sync.dma_start(wqk[:], Wqk[blk][c])
                        for mi, m in enumerate(act_m):
                            na = nact[m]
                            for ti, tname in enumerate("qk"):
                                acc = psum(qk_ctr[0] % 3)
                                qk_ctr[0] += 1
                                for dk in range(NK[m]):
                                    nc.tensor.matmul(
                                        acc[:, :na],
                                        wqk[:, ti, dkoff[m] + dk, :],
                                        xt[(m, dk)][:, :na],
                                        start=(dk == 0), stop=(dk == NK[m] - 1),
                                        skip_group_check=True)
                                t = qkpool.tile([128, BLK], BF16,
                                                tag=f"{tname}{m}_{c % 2}",
                                                name=f"{tname}{m}")
                                nc.scalar.activation(
                                    t[:, :na], acc[:, :na], AF.Identity,
                                    bias=betaqk[:, m, ti, c:c + 1])
                                if na < BLK:
                                    nc.gpsimd.memset(t[:, na:].bitcast(F32), 0.0)
                                qk_t[(tname, m)] = t
                        return qk_t

                    def emit_scores(c, qk_t):
                        for pi, (m, w, r) in enumerate(act_pairs):
                            i = c_sc[w]
                            c_sc[w] += 1
                            # first matmul of a window must cover the full
                            # width (start=True zeroes the tail); later ones
                            # only need the region where q*k can be nonzero
                            nw = BLK if i == 0 else min(nact[m], nact[r])
                            prod = prpool.tile([128, BLK], BF16, bufs=1,
                                               tag=f"prod{pi % 4}", name="prod")
                            nc.vector.tensor_mul(
                                prod[:, :nw], qk_t[("q", m)][:, :nw],
                                qk_t[("k", r)][:, :nw])
                            off = 62 - (16 * m + 2 * c)
                            nc.tensor.matmul(
                                sc_ps[w][:, :nw], selw[:, off:off + 64],
                                prod[:, :nw],
                                start=(i == 0), stop=(i == n_sc[w] - 1),
                                skip_group_check=True)

                    prev = emit_qk(0)
                    for c in range(1, NCH):
                        cur = emit_qk(c)
                        emit_scores(c - 1, prev)
                        prev = cur
                    emit_scores(NCH - 1, prev)
                    if pending_D:
                        emit_stage_D(*pending_D.pop(0))

                    # ---------- softmax ----------
                    s_sb = []
                    for w in range(WIN):
                        t = smpool.tile([64, BLK], F32, tag=f"s{w}")
                        if sc_ps[w] is None:
                            nc.gpsimd.memset(t[:], 0.0)
                        else:
                            nc.vector.tensor_copy(t[:], sc_ps[w][:])
                        s_sb.append(t)
                    mx = smpool.tile([64, BLK], F32, tag="mx")
                    nc.vector.tensor_tensor(mx[:], s_sb[0][:], s_sb[1][:], op=ALU.max)
                    nc.vector.tensor_tensor(mx[:], mx[:], s_sb[2][:], op=ALU.max)
                    attn = []
                    for w in range(WIN):
                        nc.vector.tensor_tensor(s_sb[w][:], s_sb[w][:], mx[:],
                                                op=ALU.subtract)
                        a = smpool.tile([64, BLK], BF16, tag=f"at{w}")
                        nc.scalar.activation(a[:], s_sb[w][:], AF.Exp, scale=scale)
                        attn.append(a)
                    den = smpool.tile([64, BLK], F32, tag="mx")
                    nc.vector.tensor_add(den[:], attn[0][:], attn[1][:])
                    nc.vector.tensor_add(den[:], den[:], attn[2][:])
                    rec = smpool.tile([64, BLK], F32R, tag="rec")
                    with nc.allow_low_precision(reason="fp32r attn weights"):
                        nc.vector.reciprocal(rec[:], den[:])
                    for w in range(WIN):
                        nc.vector.tensor_mul(attn[w][:], attn[w][:], rec[:])

                    # ---------- pass 2: v, A16, Abc, fused ----------
                    act_r = [r for r in range(M) if nact[r] > 0]
                    fz = fzpool.tile([128, NCH, BLK], BF16, tag="fz")
                    v_ctr = [0]

                    def emit_v(c):
                        v_t = {}
                        wv = wqpool.tile([128, SNK[blk], 128], BF16,
                                         tag=f"wv{blk % 2}", name="wv")
                        nc.sync.dma_start(wv[:], Wvf[blk][c])
                        for mi, m in enumerate(act_m):
                            na = nact[m]
                            acc = psum(v_ctr[0] % 3)
                            v_ctr[0] += 1
                            for dk in range(NK[m]):
                                nc.tensor.matmul(
                                    acc[:, :na], wv[:, dkoff[m] + dk, :],
                                    xt[(m, dk)][:, :na],
                                    start=(dk == 0), stop=(dk == NK[m] - 1),
                                    skip_group_check=True)
                            t = qkpool.tile([128, BLK], BF16,
                                            tag=f"q{m}_{c % 2}", name="vt")
                            nc.scalar.activation(
                                t[:, :na], acc[:, :na], AF.Identity,
                                bias=betav[:, m, c:c + 1])
                            if na < BLK:
                                nc.gpsimd.memset(t[:, na:].bitcast(F32), 0.0)
                            v_t[m] = t
                        return v_t

                    def emit_fused(c, v_t, a16sb):
                        ab_ps = {}
                        for ri, r in enumerate(act_r):
                            ab = psum(3 + ri)
                            nc.tensor.matmul(
                                ab[:], selB[:, c, :], a16sb[:, r, :],
                                start=True, stop=True,
                                skip_group_check=True)
                            ab_ps[r] = ab
                        r0 = act_r[0]
                        accv = prpool.tile([128, BLK], F32, tag="f0",
                                           name="accv")
                        nc.vector.tensor_mul(accv[:], ab_ps[r0][:], v_t[r0][:])
                        if len(act_r) == 1:
                            nc.vector.tensor_copy(fz[:, c, :], accv[:])
                        for j, r in enumerate(act_r[1:]):
                            tmp = prpool.tile([128, BLK], F32, tag="f1",
                                              name="tmp")
                            nc.vector.tensor_mul(tmp[:], ab_ps[r][:], v_t[r][:])
                            last = (j == len(act_r) - 2)
                            nc.vector.tensor_add(
                                fz[:, c, :] if last else accv[:],
                                accv[:], tmp[:])

                    vbuf = {0: emit_v(0)}
                    if NCH > 1:
                        vbuf[1] = emit_v(1)

                    # A16 = per-source summed attn (waits on softmax; emitted
                    # after two v chunks so the PE stream has work meanwhile)
                    a16sb = smpool.tile([16, M, BLK], BF16, tag="a16sb")
                    for ri, r in enumerate(act_r):
                        a16 = psum(3 + (ri % 2), (16, BLK))
                        srcs = SRC[r]
                        for i, (m, w) in enumerate(srcs):
                            nc.tensor.matmul(
                                a16[:], selA[:, PAIR_IDX[(m, w)], :],
                                attn[w][:],
                                start=(i == 0), stop=(i == len(srcs) - 1),
                                skip_group_check=True)
                        nc.scalar.activation(a16sb[:, r, :], a16[:], AF.Identity)

                    for c in range(NCH):
                        emit_fused(c, vbuf.pop(c), a16sb)
                        if c + 2 < NCH:
                            vbuf[c + 2] = emit_v(c + 2)

                    # ---------- stage D (deferred) ----------
                    def emit_stage_D(fz, p0):
                        yo = yopool.tile([128, NCH, BLK], F32, tag="yo",
                                         bufs=1)
                        for half in range(2):
                            wsl = wopool.tile([128, NCH // 2, NCH, 128], BF16,
                                              tag="wo", name="wsld")
                            nc.sync.dma_start(
                                wsl[:], WoT[:, half * (NCH // 2):
                                            (half + 1) * (NCH // 2)])
                            for dci in range(NCH // 2):
                                dc = half * (NCH // 2) + dci
                                acc = psum(3 + dc % 2)
                                for dk in range(NCH):
                                    nc.tensor.matmul(
                                        acc[:], wsl[:, dci, dk, :],
                                        fz[:, dk, :],
                                        start=(dk == 0), stop=(dk == NCH - 1),
                                        skip_group_check=True)
                                nc.scalar.activation(
                                    yo[:, dc, :], acc[:],
                                    AF.Identity, bias=bo[:, dc:dc + 1])
                        nc.scalar.dma_start(yT[:, :, p0:p0 + BLK], yo[:])

                    pending_D.append((fz, p0))

                    if blk == NBLK - 1:
                        while pending_D:
                            emit_stage_D(*pending_D.pop(0))

    nc.compile()
    _BUILD_CACHE[key] = nc
    return nc


def make_selw():
    sw = np.zeros((128, 127), np.float32)
    for p in range(128):
        sw[p, 62 + p // 64] = 1.0
    return sw


def make_selA():
    sa = np.zeros((64, M * WIN, 16), np.float32)
    for m in range(M):
        for w in range(WIN):
            for h in range(16):
                sa[16 * m + h, m * WIN + w, h] = 1.0
    return sa


def make_selB():
    sb = np.zeros((16, NCH, 128), np.float32)
    for c in range(NCH):
        for j in range(128):
            sb[2 * c + j // 64, c, j] = 0.25
    return sb


def _vec_tile(v):
    return np.ascontiguousarray(np.asarray(v, np.float32).reshape(NCH, 128).T)


def _wf_tiles(Wf):
    """[D(out), dim(in)] fused weight -> [c, p(din%128), dk, j(dout%128)] bf16."""
    wt = np.asarray(Wf, np.float32).T                  # [din, dout]
    nk = wt.shape[0] // 128
    wt = wt.reshape(nk, 128, NCH, 128)                 # [dk, p, c, j]
    return wt.transpose(2, 1, 0, 3)                    # [c, p, dk, j]


def prepare_in_maps(inputs):
    names = [mm[0] for mm in MOD]
    emb = np.asarray(inputs["mod_emb"], np.float32)
    Wp = {pn: np.asarray(inputs[f"W{pn}"], np.float32) for pn in "qkvo"}
    bp = {pn: np.asarray(inputs[f"b{pn}"], np.float32) for pn in "qkvo"}

    shared = {}
    betaqk = np.zeros((128, M, 2, NCH), np.float32)
    betav = np.zeros((128, M, NCH), np.float32)
    tq, tk, tv = {}, {}, {}
    for i, nm in enumerate(names):
        Wm = np.asarray(inputs[f"W_{nm}"], np.float32)     # [D, dim]
        bm = np.asarray(inputs[f"b_{nm}"], np.float32) + emb[i]
        tq[i] = _wf_tiles(Wp["q"] @ Wm)
        tk[i] = _wf_tiles(Wp["k"] @ Wm)
        tv[i] = _wf_tiles(Wp["v"] @ Wm)
        betaqk[:, i, 0, :] = _vec_tile(Wp["q"] @ bm + bp["q"])
        betaqk[:, i, 1, :] = _vec_tile(Wp["k"] @ bm + bp["k"])
        betav[:, i, :] = _vec_tile(Wp["v"] @ bm + bp["v"])
    shared["betaqk"] = betaqk
    shared["betav"] = betav

    # merged per-block weight tensors, concatenated over active modalities
    for b in range(NBLK):
        act = [m for m in range(M) if n_active(m, b) > 0]
        shared[f"Wqk{b}"] = np.ascontiguousarray(np.concatenate(
            [np.stack([tq[m], tk[m]], axis=2) for m in act],
            axis=3)).astype(ml_dtypes.bfloat16)
        shared[f"Wvf{b}"] = np.ascontiguousarray(np.concatenate(
            [tv[m] for m in act], axis=2)).astype(ml_dtypes.bfloat16)

    wo = Wp["o"].T.reshape(NCH, 128, NCH, 128)             # [dk, p, c, j]
    shared["WoT"] = np.ascontiguousarray(
        wo.transpose(1, 2, 0, 3)).astype(ml_dtypes.bfloat16)
    shared["bo"] = _vec_tile(bp["o"])
    shared["selw"] = make_selw().astype(ml_dtypes.bfloat16)
    shared["selA"] = make_selA().astype(ml_dtypes.bfloat16)
    shared["selB"] = make_selB().astype(ml_dtypes.bfloat16)

    in_maps = []
    for core in range(8):
        b, par = core // 2, core % 2
        im = dict(shared)
        for i, nm in enumerate(names):
            x = np.asarray(inputs[nm], np.float32)[b, par::2][:NLOC[i]]
            xt = x.T.reshape(NK[i], 128, NLOC[i]).transpose(1, 0, 2)
            im[f"xT{i}"] = np.ascontiguousarray(xt).astype(ml_dtypes.bfloat16)
        in_maps.append(im)
    return in_maps


def kernel(**inputs):
    inputs = {k: np.asarray(v) for k, v in inputs.items()}
    scale = float(1.0 / (np.sqrt(HD) * abs(float(inputs["temperature"]))))
    nc = build(scale, repeat=1)
    in_maps = prepare_in_maps(inputs)
    res = run_bass_kernel_spmd(nc, in_maps, list(range(8)))
    out = np.zeros((B, S, D), np.float32)
    for core in range(8):
        b, par = core // 2, core % 2
        y = res.results[core]["yT"].transpose(1, 0, 2).reshape(D, NPOS)
        out[b, par::2, :] = y.T
    return out


# revision 17
# speedup vs baseline: 1.0194x; 1.0194x over previous
"""Trainium2 Bass kernel for nn_CantorModalityFusion.

Sharding: 8 cores = (batch b in 0..3) x (position parity in 0..1).
Each core handles batch b, positions s = par, par+2, ... (1024 positions).
The computation is per-(b, s) independent -> no collectives.

v2: the per-modality input projection is folded into the QKV weights on
the host (Wf = Wp @ W_m, beta = Wp @ (b_m + emb_m) + b_p), so the device
computes q/k/v for each modality directly from the raw modality input:
  q_m = Wfq_m @ x_m + beta  (contraction over dim_m instead of D)
This removes the separate projection stage and cuts PE work ~35%.
Weights and x stream in bf16 (fp32 PSUM accumulation); q/k/v, scores,
softmax, fused accumulation and the output projection stay fp32.

Pipeline per 512-position block:
  B1: q.T/k.T per feature chunk from x; s_w += sel.T @ (q*k)     [PE+DVE]
  SM: softmax over the 3 routed windows                          [DVE+ACT]
  B2: v.T per chunk; A16_r = sum attn; Abc = bcast(A16);
      fused.T[c] = sum_r Abc_r * v.T[r]                          [PE+DVE]
  D:  y.T = Wo.T.T @ fused.T (+ bo)  (deferred into next block)  [PE+ACT]
"""

import sys

import numpy as np

sys.path.insert(0, "/opt/trn_rl_repo")

import ml_dtypes

import concourse.bacc as bacc
import concourse.mybir as mybir
from concourse import tile
from concourse.bass_utils import run_bass_kernel_spmd

F32 = mybir.dt.float32
F32R = mybir.dt.float32r
BF16 = mybir.dt.bfloat16
AF = mybir.ActivationFunctionType
ALU = mybir.AluOpType

B, S, D, H, HD = 4, 2048, 1024, 16, 64
M, WIN = 4, 3
MOD = [("text", 768, 2048), ("image", 1024, 1024), ("audio", 512, 1500), ("video", 2048, 512)]
ROUTES = [[0, 1, 2], [0, 1, 2], [2, 3, 0], [3, 2, 0]]
PAIRS = [(m, w, ROUTES[m][w]) for m in range(M) for w in range(WIN)]
SRC = {r: [(m, w) for (m, w, rr) in PAIRS if rr == r] for r in range(M)}
PAIR_IDX = {(m, w): m * WIN + w for m in range(M) for w in range(WIN)}

NPOS = S // 2
BLK = 512
NBLK = NPOS // BLK
NCH = D // 128                           # 8 output feature chunks
NLOC = [sl // 2 for (_, _, sl) in MOD]   # 1024, 512, 750, 256
NK = [dim // 128 for (_, dim, _) in MOD]  # 6, 8, 4, 16 input chunks

_BUILD_CACHE = {}


def n_active(m, blk):
    return max(0, min(BLK, NLOC[m] - blk * BLK))


def build(scale, repeat=1):
    key = (float(scale), repeat)
    if key in _BUILD_CACHE:
        return _BUILD_CACHE[key]
    nc = bacc.Bacc("TRN2", target_bir_lowering=False, debug=False)

    xT = [nc.dram_tensor(f"xT{m}", [128, NK[m], NLOC[m]], BF16,
                         kind="ExternalInput") for m in range(M)]
    # fused q/k weights, merged over the modalities active in each block:
    # [c, p(din%128), ti(q/k), dk(concat over act_m), j(dout%128)]
    SNK = [sum(NK[m] for m in range(M) if n_active(m, b) > 0)
           for b in range(NBLK)]
    Wqk = [nc.dram_tensor(f"Wqk{b}", [NCH, 128, 2, SNK[b], 128], BF16,
                          kind="ExternalInput") for b in range(NBLK)]
    Wvf = [nc.dram_tensor(f"Wvf{b}", [NCH, 128, SNK[b], 128], BF16,
                          kind="ExternalInput") for b in range(NBLK)]
    WoT = nc.dram_tensor("WoT", [128, NCH, NCH, 128], BF16, kind="ExternalInput")
    betaqk_d = nc.dram_tensor("betaqk", [128, M, 2, NCH], F32, kind="ExternalInput")
    betav_d = nc.dram_tensor("betav", [128, M, NCH], F32, kind="ExternalInput")
    bo_d = nc.dram_tensor("bo", [128, NCH], F32, kind="ExternalInput")
    selw_d = nc.dram_tensor("selw", [128, 127], BF16, kind="ExternalInput")
    selA_d = nc.dram_tensor("selA", [64, M * WIN, 16], BF16, kind="ExternalInput")
    selB_d = nc.dram_tensor("selB", [16, NCH, 128], BF16, kind="ExternalInput")
    yT = nc.dram_tensor("yT", [128, NCH, NPOS], F32, kind="ExternalOutput")

    with tile.TileContext(nc) as tc:
        with (
            tc.tile_pool(name="const", bufs=1) as cpool,
            tc.tile_pool(name="wq", bufs=2) as wqpool,
            tc.tile_pool(name="wo", bufs=2) as wopool,
            tc.tile_pool(name="xt", bufs=1) as xtpool,
            tc.tile_pool(name="qk", bufs=1) as qkpool,
            tc.tile_pool(name="pr", bufs=2) as prpool,
            tc.tile_pool(name="sm", bufs=1) as smpool,
            tc.tile_pool(name="fz", bufs=2) as fzpool,
            tc.tile_pool(name="yo", bufs=2) as yopool,
            tc.tile_pool(name="ps", bufs=1, space="PSUM") as pspool,
        ):
            def psum(i, shape=(128, BLK)):
                return pspool.tile(list(shape), F32, tag=f"a{i}", name=f"ps_a{i}")

            # ---- constants ----
            selw = cpool.tile([128, 127], BF16, tag="selw")
            nc.sync.dma_start(selw[:], selw_d[:])
            selA = cpool.tile([64, M * WIN, 16], BF16, tag="selA")
            nc.sync.dma_start(selA[:], selA_d[:])
            selB = cpool.tile([16, NCH, 128], BF16, tag="selB")
            nc.sync.dma_start(selB[:], selB_d[:])
            betaqk = cpool.tile([128, M, 2, NCH], F32, tag="betaqk")
            nc.sync.dma_start(betaqk[:], betaqk_d[:])
            betav = cpool.tile([128, M, NCH], F32, tag="betav")
            nc.sync.dma_start(betav[:], betav_d[:])
            bo = cpool.tile([128, NCH], F32, tag="bo")
            nc.sync.dma_start(bo[:], bo_d[:])

            import contextlib
            rep_cm = (tc.For_i(0, repeat, 1,
                               hint_engines=(mybir.EngineType.PE,
                                             mybir.EngineType.Activation,
                                             mybir.EngineType.DVE,
                                             mybir.EngineType.SP,
                                             mybir.EngineType.Pool))
                      if repeat > 1 else contextlib.nullcontext())
            pending_D = []
            with rep_cm:
                for blk in range(NBLK):
                    nact = [n_active(m, blk) for m in range(M)]
                    act_m = [m for m in range(M) if nact[m] > 0]
                    act_pairs = [(m, w, r) for (m, w, r) in PAIRS
                                 if nact[m] > 0 and nact[r] > 0]
                    p0 = blk * BLK

                    # ---------- x tiles (bf16, reused by q, k and v) ----------
                    xt = {}
                    dkoff = {}
                    off = 0
                    for mi, m in enumerate(act_m):
                        na = nact[m]
                        dkoff[m] = off
                        off += NK[m]
                        t = xtpool.tile([128, NK[m], BLK], BF16,
                                        tag=f"x{m}_{blk % 2}")
                        nc.scalar.dma_start(t[:, :, :na], xT[m][:, :, p0:p0 + na])
                        for dk in range(NK[m]):
                            xt[(m, dk)] = t[:, dk, :]

                    # ---------- pass 1: q, k, scores ----------
                    n_sc = {w: sum(1 for (m, w2, r) in act_pairs if w2 == w) * NCH
                            for w in range(WIN)}
                    c_sc = {w: 0 for w in range(WIN)}
                    sc_ps = [psum(5 + w, (64, BLK)) if n_sc[w] > 0 else None
                             for w in range(WIN)]
                    qk_ctr = [0]

                    def emit_qk(c):
                        qk_t = {}
                        wqk = wqpool.tile([128, 2, SNK[blk], 128], BF16,
                                          tag=f"wqk{blk % 2}", name="wqk")
                        hs = SNK[blk] // 2
                        nc.sync.dma_start(wqk[:, :, :hs, :],
                                          Wqk[blk][c][:, :, :hs, :])
                        nc.gpsimd.dma_start(wqk[:, :, hs:, :],
                                            Wqk[blk][c][:, :, hs:, :])
                        for mi, m in enumerate(act_m):
                            na = nact[m]
                            for ti, tname in enumerate("qk"):
                                acc = psum(qk_ctr[0] % 3)
                                qk_ctr[0] += 1
                                for dk in range(NK[m]):
                                    nc.tensor.matmul(
                                        acc[:, :na],
                                        wqk[:, ti, dkoff[m] + dk, :],
                                        xt[(m, dk)][:, :na],
                                        start=(dk == 0), stop=(dk == NK[m] - 1),
                                        skip_group_check=True)
                                t = qkpool.tile([128, BLK], BF16,
                                                tag=f"{tname}{m}_{c % 2}",
                                                name=f"{tname}{m}")
                                nc.scalar.activation(
                                    t[:, :na], acc[:, :na], AF.Identity,
                                    bias=betaqk[:, m, ti, c:c + 1])
                                if na < BLK:
                                    nc.gpsimd.memset(t[:, na:].bitcast(F32), 0.0)
                                qk_t[(tname, m)] = t
                        return qk_t

                    def emit_scores(c, qk_t):
                        for pi, (m, w, r) in enumerate(act_pairs):
                            i = c_sc[w]
                            c_sc[w] += 1
                            # first matmul of a window must cover the full
                            # width (start=True zeroes the tail); later ones
                            # only need the region where q*k can be nonzero
                            nw = BLK if i == 0 else min(nact[m], nact[r])
                            prod = prpool.tile([128, BLK], BF16, bufs=1,
                                               tag=f"prod{pi % 4}", name="prod")
                            nc.vector.tensor_mul(
                                prod[:, :nw], qk_t[("q", m)][:, :nw],
                                qk_t[("k", r)][:, :nw])
                            off = 62 - (16 * m + 2 * c)
                            nc.tensor.matmul(
                                sc_ps[w][:, :nw], selw[:, off:off + 64],
                                prod[:, :nw],
                                start=(i == 0), stop=(i == n_sc[w] - 1),
                                skip_group_check=True)

                    prev = emit_qk(0)
                    for c in range(1, NCH):
                        cur = emit_qk(c)
                        emit_scores(c - 1, prev)
                        prev = cur
                    emit_scores(NCH - 1, prev)
                    if pending_D:
                        emit_stage_D(*pending_D.pop(0))

                    # ---------- softmax ----------
                    s_sb = []
                    for w in range(WIN):
                        t = smpool.tile([64, BLK], F32, tag=f"s{w}")
                        if sc_ps[w] is None:
                            nc.gpsimd.memset(t[:], 0.0)
                        else:
                            nc.vector.tensor_copy(t[:], sc_ps[w][:])
                        s_sb.append(t)
                    mx = smpool.tile([64, BLK], F32, tag="mx")
                    nc.vector.tensor_tensor(mx[:], s_sb[0][:], s_sb[1][:], op=ALU.max)
                    nc.vector.tensor_tensor(mx[:], mx[:], s_sb[2][:], op=ALU.max)
                    attn = []
                    for w in range(WIN):
                        nc.vector.tensor_tensor(s_sb[w][:], s_sb[w][:], mx[:],
                                                op=ALU.subtract)
                        a = smpool.tile([64, BLK], BF16, tag=f"at{w}")
                        nc.scalar.activation(a[:], s_sb[w][:], AF.Exp, scale=scale)
                        attn.append(a)
                    den = smpool.tile([64, BLK], F32, tag="mx")
                    nc.vector.tensor_add(den[:], attn[0][:], attn[1][:])
                    nc.vector.tensor_add(den[:], den[:], attn[2][:])
                    rec = smpool.tile([64, BLK], F32R, tag="rec")
                    with nc.allow_low_precision(reason="fp32r attn weights"):
                        nc.vector.reciprocal(rec[:], den[:])
                    for w in range(WIN):
                        nc.vector.tensor_mul(attn[w][:], attn[w][:], rec[:])

                    # ---------- pass 2: v, A16, Abc, fused ----------
                    act_r = [r for r in range(M) if nact[r] > 0]
                    fz = fzpool.tile([128, NCH, BLK], BF16, tag="fz")
                    v_ctr = [0]

                    def emit_v(c):
                        v_t = {}
                        wv = wqpool.tile([128, SNK[blk], 128], BF16,
                                         tag=f"wv{blk % 2}", name="wv")
                        hv = SNK[blk] // 2
                        nc.sync.dma_start(wv[:, :hv, :], Wvf[blk][c][:, :hv, :])
                        nc.gpsimd.dma_start(wv[:, hv:, :], Wvf[blk][c][:, hv:, :])
                        for mi, m in enumerate(act_m):
                            na = nact[m]
                            acc = psum(v_ctr[0] % 3)
                            v_ctr[0] += 1
                            for dk in range(NK[m]):
                                nc.tensor.matmul(
                                    acc[:, :na], wv[:, dkoff[m] + dk, :],
                                    xt[(m, dk)][:, :na],
                                    start=(dk == 0), stop=(dk == NK[m] - 1),
                                    skip_group_check=True)
                            t = qkpool.tile([128, BLK], BF16,
                                            tag=f"q{m}_{c % 2}", name="vt")
                            nc.scalar.activation(
                                t[:, :na], acc[:, :na], AF.Identity,
                                bias=betav[:, m, c:c + 1])
                            if na < BLK:
                                nc.gpsimd.memset(t[:, na:].bitcast(F32), 0.0)
                            v_t[m] = t
                        return v_t

                    def emit_fused(c, v_t, a16sb):
                        ab_ps = {}
                        for ri, r in enumerate(act_r):
                            ab = psum(3 + ri)
                            nc.tensor.matmul(
                                ab[:], selB[:, c, :], a16sb[:, r, :],
                                start=True, stop=True,
                                skip_group_check=True)
                            ab_ps[r] = ab
                        r0 = act_r[0]
                        accv = prpool.tile([128, BLK], F32, tag="f0",
                                           name="accv")
                        nc.vector.tensor_mul(accv[:], ab_ps[r0][:], v_t[r0][:])
                        if len(act_r) == 1:
                            nc.vector.tensor_copy(fz[:, c, :], accv[:])
                        for j, r in enumerate(act_r[1:]):
                            tmp = prpool.tile([128, BLK], F32, tag="f1",
                                              name="tmp")
                            nc.vector.tensor_mul(tmp[:], ab_ps[r][:], v_t[r][:])
                            last = (j == len(act_r) - 2)
                            nc.vector.tensor_add(
                                fz[:, c, :] if last else accv[:],
                                accv[:], tmp[:])

                    vbuf = {0: emit_v(0)}
                    if NCH > 1:
                        vbuf[1] = emit_v(1)

                    # A16 = per-source summed attn (waits on softmax; emitted
                    # after two v chunks so the PE stream has work meanwhile)
                    a16sb = smpool.tile([16, M, BLK], BF16, tag="a16sb")
                    for ri, r in enumerate(act_r):
                        a16 = psum(3 + (ri % 2), (16, BLK))
                        srcs = SRC[r]
                        for i, (m, w) in enumerate(srcs):
                            nc.tensor.matmul(
                                a16[:], selA[:, PAIR_IDX[(m, w)], :],
                                attn[w][:],
                                start=(i == 0), stop=(i == len(srcs) - 1),
                                skip_group_check=True)
                        nc.scalar.activation(a16sb[:, r, :], a16[:], AF.Identity)

                    for c in range(NCH):
                        emit_fused(c, vbuf.pop(c), a16sb)
                        if c + 2 < NCH:
                            vbuf[c + 2] = emit_v(c + 2)

                    # ---------- stage D (deferred) ----------
                    def emit_stage_D(fz, p0):
                        yo = yopool.tile([128, NCH, BLK], F32, tag="yo",
                                         bufs=1)
                        for half in range(2):
                            wsl = wopool.tile([128, NCH // 2, NCH, 128], BF16,
                                              tag="wo", name="wsld")
                            deng = nc.sync if half == 0 else nc.gpsimd
                            deng.dma_start(
                                wsl[:], WoT[:, half * (NCH // 2):
                                            (half + 1) * (NCH // 2)])
                            for dci in range(NCH // 2):
                                dc = half * (NCH // 2) + dci
                                acc = psum(3 + dc % 2)
                                for dk in range(NCH):
                                    nc.tensor.matmul(
                                        acc[:], wsl[:, dci, dk, :],
                                        fz[:, dk, :],
                                        start=(dk == 0), stop=(dk == NCH - 1),
                                        skip_group_check=True)
                                nc.scalar.activation(
                                    yo[:, dc, :], acc[:],
                                    AF.Identity, bias=bo[:, dc:dc + 1])
                        nc.sync.dma_start(yT[:, :, p0:p0 + BLK], yo[:])

                    pending_D.append((fz, p0))

                    if blk == NBLK - 1:
                        while pending_D:
                            emit_stage_D(*pending_D.pop(0))

    nc.compile()
    _BUILD_CACHE[key] = nc
    return nc


def make_selw():
    sw = np.zeros((128, 127), np.float32)
    for p in range(128):
        sw[p, 62 + p // 64] = 1.0
    return sw


def make_selA():
    sa = np.zeros((64, M * WIN, 16), np.float32)
    for m in range(M):
        for w in range(WIN):
            for h in range(16):
                sa[16 * m + h, m * WIN + w, h] = 1.0
    return sa


def make_selB():
    sb = np.zeros((16, NCH, 128), np.float32)
    for c in range(NCH):
        for j in range(128):
            sb[2 * c + j // 64, c, j] = 0.25
    return sb


def _vec_tile(v):
    return np.ascontiguousarray(np.asarray(v, np.float32).reshape(NCH, 128).T)


def _wf_tiles(Wf):
    """[D(out), dim(in)] fused weight -> [c, p(din%128), dk, j(dout%128)] bf16."""
    wt = np.asarray(Wf, np.float32).T                  # [din, dout]
    nk = wt.shape[0] // 128
    wt = wt.reshape(nk, 128, NCH, 128)                 # [dk, p, c, j]
    return wt.transpose(2, 1, 0, 3)                    # [c, p, dk, j]


def prepare_in_maps(inputs):
    names = [mm[0] for mm in MOD]
    emb = np.asarray(inputs["mod_emb"], np.float32)
    Wp = {pn: np.asarray(inputs[f"W{pn}"], np.float32) for pn in "qkvo"}
    bp = {pn: np.asarray(inputs[f"b{pn}"], np.float32) for pn in "qkvo"}

    shared = {}
    betaqk = np.zeros((128, M, 2, NCH), np.float32)
    betav = np.zeros((128, M, NCH), np.float32)
    tq, tk, tv = {}, {}, {}
    for i, nm in enumerate(names):
        Wm = np.asarray(inputs[f"W_{nm}"], np.float32)     # [D, dim]
        bm = np.asarray(inputs[f"b_{nm}"], np.float32) + emb[i]
        tq[i] = _wf_tiles(Wp["q"] @ Wm)
        tk[i] = _wf_tiles(Wp["k"] @ Wm)
        tv[i] = _wf_tiles(Wp["v"] @ Wm)
        betaqk[:, i, 0, :] = _vec_tile(Wp["q"] @ bm + bp["q"])
        betaqk[:, i, 1, :] = _vec_tile(Wp["k"] @ bm + bp["k"])
        betav[:, i, :] = _vec_tile(Wp["v"] @ bm + bp["v"])
    shared["betaqk"] = betaqk
    shared["betav"] = betav

    # merged per-block weight tensors, concatenated over active modalities
    for b in range(NBLK):
        act = [m for m in range(M) if n_active(m, b) > 0]
        shared[f"Wqk{b}"] = np.ascontiguousarray(np.concatenate(
            [np.stack([tq[m], tk[m]], axis=2) for m in act],
            axis=3)).astype(ml_dtypes.bfloat16)
        shared[f"Wvf{b}"] = np.ascontiguousarray(np.concatenate(
            [tv[m] for m in act], axis=2)).astype(ml_dtypes.bfloat16)

    wo = Wp["o"].T.reshape(NCH, 128, NCH, 128)             # [dk, p, c, j]
    shared["WoT"] = np.ascontiguousarray(
        wo.transpose(1, 2, 0, 3)).astype(ml_dtypes.bfloat16)
    shared["bo"] = _vec_tile(bp["o"])
    shared["selw"] = make_selw().astype(ml_dtypes.bfloat16)
    shared["selA"] = make_selA().astype(ml_dtypes.bfloat16)
    shared["selB"] = make_selB().astype(ml_dtypes.bfloat16)

    in_maps = []
    for core in range(8):
        b, par = core // 2, core % 2
        im = dict(shared)
        for i, nm in enumerate(names):
            x = np.asarray(inputs[nm], np.float32)[b, par::2][:NLOC[i]]
            xt = x.T.reshape(NK[i], 128, NLOC[i]).transpose(1, 0, 2)
            im[f"xT{i}"] = np.ascontiguousarray(xt).astype(ml_dtypes.bfloat16)
        in_maps.append(im)
    return in_maps


def kernel(**inputs):
    inputs = {k: np.asarray(v) for k, v in inputs.items()}
    scale = float(1.0 / (np.sqrt(HD) * abs(float(inputs["temperature"]))))
    nc = build(scale, repeat=1)
    in_maps = prepare_in_maps(inputs)
    res = run_bass_kernel_spmd(nc, in_maps, list(range(8)))
    out = np.zeros((B, S, D), np.float32)
    for core in range(8):
        b, par = core // 2, core % 2
        y = res.results[core]["yT"].transpose(1, 0, 2).reshape(D, NPOS)
        out[b, par::2, :] = y.T
    return out
